# revision 4
# baseline (speedup 1.0000x reference)
"""GTN (graph transformer network) forward on 8 Trainium2 cores.

Math (mirrors the reference, normalizations folded):
  A[t] = dense adjacency from edge lists           (host, bincount)
  A1 = softmax(w_l0_c1) . A ; A2 = softmax(w_l0_c2) . A ; A3 = softmax(w_l1_c1) . A
  H1 = A1 @ A2                                     (device)
  U  = H1 @ A3                                     (device)
  Because all entries are >= 0 and row scaling commutes through matmul,
  rownorm(rownorm(H1) @ A3) == rownorm(U).  With XW1 = [X @ gcn_w | 1],
  Z = U @ XW1 gives both U @ XW (cols :128) and rowsum(U) (col 128) so the
  row normalization can be applied on the host after the fact.
  y = relu(Z[:, :128]/rowsum + b) -> channel concat -> target gather -> linear.

Sharding: 1D row shard, 512 rows per core, both channels per core.
Device computes, per channel:
  H1T = matmul(lhsT=A2 tiles, rhs=A1_rows^T)   [4096 x 512] (stays in SBUF)
  UT  = matmul(lhsT=A3 tiles, rhs=H1T)         consumed tile-by-tile
  Z  += matmul(lhsT=UT tile,  rhs=XW1 chunk)   accumulated in PSUM over j
All matmuls run in float32r (full-rate fp32 mode, ~1e-4 rel err).
"""

import os
import numpy as np
from contextlib import ExitStack

NUM_EDGE = 5
C = 2
N = 4096
W_IN = 512
W_OUT = 128
NCORES = 8
P = 128
R = N // NCORES          # 512 rows per core
NK = N // P              # 32 chunks of the contraction dims
NI = R // P              # 4 row subtiles per core
DOUT = W_OUT + 4         # 132: XW cols + ones col + zero pad (f32r needs even/4-aligned moving dim)

_NC_CACHE = {}
LAST_EXEC_NS = None


def _build_nc():
    import concourse.tile as tile
    from concourse import bacc, mybir

    nc = bacc.Bacc("TRN2", target_bir_lowering=False, debug=False,
                   num_devices=NCORES)
    f32 = mybir.dt.float32
    f32r = mybir.dt.float32r

    a1t = nc.dram_tensor("a1t", [C, N, R], f32, kind="ExternalInput").ap()
    a2 = nc.dram_tensor("a2", [C, N, N], f32, kind="ExternalInput").ap()
    a3 = nc.dram_tensor("a3", [C, N, N], f32, kind="ExternalInput").ap()
    xw = nc.dram_tensor("xw", [N, DOUT], f32, kind="ExternalInput").ap()
    z = nc.dram_tensor("z", [C, R, DOUT], f32, kind="ExternalOutput").ap()

    with tile.TileContext(nc) as tc, ExitStack() as ctx:
        bigp = ctx.enter_context(tc.tile_pool(name="bigp", bufs=1))
        a1p = ctx.enter_context(tc.tile_pool(name="a1p", bufs=1))
        h1p = ctx.enter_context(tc.tile_pool(name="h1p", bufs=1))
        stripp = ctx.enter_context(tc.tile_pool(name="stripp", bufs=2))
        utp = ctx.enter_context(tc.tile_pool(name="utp", bufs=3))
        zsbp = ctx.enter_context(tc.tile_pool(name="zsbp", bufs=4))
        psH = ctx.enter_context(tc.tile_pool(name="psH", bufs=2, space="PSUM"))
        psU = ctx.enter_context(tc.tile_pool(name="psU", bufs=2, space="PSUM"))
        psZ = ctx.enter_context(tc.tile_pool(name="psZ", bufs=4, space="PSUM"))

        # xw_sb[p, k*DOUT + d] = xw[P*k + p, d]; loaded once, reused by both channels
        xw_sb = bigp.tile([P, NK * DOUT], f32r)
        nc.gpsimd.dma_start(
            xw_sb[:].rearrange("p (k d) -> p k d", k=NK),
            xw.rearrange("(k p) d -> p k d", p=P))

        for c in range(C):
            # a1_sb[p, k*R + i] = A1rowsT[P*k + p, i]
            a1_sb = a1p.tile([P, NK * R], f32r)
            nc.gpsimd.dma_start(
                a1_sb[:].rearrange("p (k i) -> p k i", k=NK),
                a1t[c].rearrange("(k p) i -> p k i", p=P))

            # H1T chunks: h1_sb[p, m*R + i] = H1T[P*m + p, i]
            h1_sb = h1p.tile([P, NK * R], f32r)

            for m in range(NK):
                # strip[p, k*P + j] = a2[c, P*k + p, P*m + j]
                st = stripp.tile([P, NK * P], f32r, tag="strip")
                nc.gpsimd.dma_start(
                    st[:].rearrange("p (k j) -> p k j", k=NK),
                    a2[c][:, m * P:(m + 1) * P].rearrange("(k p) j -> p k j", p=P))
                acc = psH.tile([P, R], f32)
                for k in range(NK):
                    nc.tensor.matmul(acc[:],
                                     st[:, k * P:(k + 1) * P],
                                     a1_sb[:, k * R:(k + 1) * R],
                                     start=(k == 0), stop=(k == NK - 1))
                nc.vector.tensor_copy(h1_sb[:, m * R:(m + 1) * R], acc[:])

            # MM2 (UT tiles) immediately consumed by MM3 (Z accumulation)
            zacc = [psZ.tile([P, DOUT], f32, tag="zacc", name=f"zacc_{c}_{i}")
                    for i in range(NI)]
            for j in range(NK):
                st = stripp.tile([P, NK * P], f32r, tag="strip")
                nc.gpsimd.dma_start(
                    st[:].rearrange("p (k j) -> p k j", k=NK),
                    a3[c][:, j * P:(j + 1) * P].rearrange("(k p) j -> p k j", p=P))
                uacc = psU.tile([P, R], f32)
                for k in range(NK):
                    nc.tensor.matmul(uacc[:],
                                     st[:, k * P:(k + 1) * P],
                                     h1_sb[:, k * R:(k + 1) * R],
                                     start=(k == 0), stop=(k == NK - 1))
                ut = utp.tile([P, R], f32r)
                nc.vector.tensor_copy(ut[:], uacc[:])
                for i in range(NI):
                    nc.tensor.matmul(zacc[i][:],
                                     ut[:, i * P:(i + 1) * P],
                                     xw_sb[:, j * DOUT:(j + 1) * DOUT],
                                     start=(j == 0), stop=(j == NK - 1),
                                     skip_group_check=True)
            for i in range(NI):
                zt = zsbp.tile([P, DOUT], f32)
                nc.vector.tensor_copy(zt[:], zacc[i][:])
                nc.sync.dma_start(z[c, i * P:(i + 1) * P, :], zt[:])

    nc.compile()
    return nc


def _get_nc():
    if "nc" not in _NC_CACHE:
        _NC_CACHE["nc"] = _build_nc()
    return _NC_CACHE["nc"]


def _softmax_rows(w):
    w = np.asarray(w, np.float32)
    e = np.exp(w - w.max(axis=1, keepdims=True))
    return (e / e.sum(axis=1, keepdims=True)).astype(np.float32)


def _install_ntff_hook():
    """Recreate antenv.axon_hooks if the image lacks it (profiling only)."""
    import sys
    import types
    try:
        from antenv.axon_hooks import get_axon_ntff_profile_hook  # noqa: F401
        return
    except ImportError:
        pass
    try:
        from trn_agent_boot.trn_boot import _ntff_profile_via_ctypes
        import antenv
        mod = types.ModuleType("antenv.axon_hooks")
        state = {"h": None}
        mod.set_axon_ntff_profile_hook = lambda h: state.__setitem__("h", h)
        mod.get_axon_ntff_profile_hook = lambda: state["h"]
        sys.modules["antenv.axon_hooks"] = mod
        antenv.axon_hooks = mod
        mod.set_axon_ntff_profile_hook(
            _ntff_profile_via_ctypes("/opt/axon/libaxon_pjrt.so"))
    except Exception:
        pass


def kernel(edge_index, edge_value, X, target_x, w_l0_c1, w_l0_c2, w_l1_c1,
           gcn_w, gcn_b, lin_w, lin_b):
    global LAST_EXEC_NS
    from concourse.bass_utils import run_bass_kernel_spmd

    # dense adjacency stack [NUM_EDGE, N, N], duplicate edges summed
    A = np.empty((NUM_EDGE, N * N), np.float32)
    src = np.asarray(edge_index[:, 0], np.int64)
    dst = np.asarray(edge_index[:, 1], np.int64)
    for t in range(NUM_EDGE):
        flat = src[t] * N + dst[t]
        A[t] = np.bincount(flat, weights=np.asarray(edge_value[t], np.float64),
                           minlength=N * N).astype(np.float32)

    def combo(w):
        f = _softmax_rows(w)               # [C, NUM_EDGE]
        return (f @ A).reshape(C, N, N)    # [C, N, N]

    A1 = combo(w_l0_c1)
    A2 = combo(w_l0_c2)
    A3 = combo(w_l1_c1)
    A = None  # free

    XW = np.asarray(X, np.float32) @ np.asarray(gcn_w, np.float32)   # [N, 128]
    xw1 = np.concatenate([XW, np.ones((N, 1), np.float32),
                      np.zeros((N, 3), np.float32)], axis=1)  # [N, 132]

    in_maps = []
    for ci in range(NCORES):
        rows = slice(ci * R, (ci + 1) * R)
        a1t_c = np.stack([np.ascontiguousarray(A1[c, rows, :].T)
                          for c in range(C)])          # [C, N, R]
        in_maps.append({"a1t": a1t_c, "a2": A2, "a3": A3, "xw": xw1})

    nc = _get_nc()
    trace = bool(int(os.environ.get("GTN_TRACE", "0")))
    import time as _time
    _t0 = _time.time()
    res = run_bass_kernel_spmd(nc, in_maps, list(range(NCORES)), trace=trace)
    _wall_ns = int((_time.time() - _t0) * 1e9)
    LAST_EXEC_NS = res.exec_time_ns if res.exec_time_ns else _wall_ns

    Z = np.concatenate([r["z"] for r in res.results], axis=1)  # [C, N, DOUT]
    s = Z[:, :, W_OUT]                                          # [C, N]
    with np.errstate(divide="ignore", invalid="ignore"):
        sinv = np.where(s == 0, 0.0, 1.0 / s).astype(np.float32)
    Hn = Z[:, :, :W_OUT] * sinv[:, :, None]                     # [C, N, 128]
    Xc = np.maximum(Hn + np.asarray(gcn_b, np.float32)[None, None, :], 0.0)
    X_ = Xc.transpose(1, 0, 2).reshape(N, C * W_OUT)            # [N, 256]
    y = X_[np.asarray(target_x, np.int64)] @ np.asarray(lin_w, np.float32)
    y = y + np.asarray(lin_b, np.float32)
    return y.astype(np.float32)
